# revision 9
# baseline (speedup 1.0000x reference)
"""Trainium2 Bass kernel for nn_DiffusionNetwork (30-step diffusion sampling).

Algorithm (algebraic restructuring + quadrature collapse + fp8 control
variate):
  1. The MLP input ``cond = z + time_embed[t]`` is independent of the scanned
     ``action``, so u = z @ W1 is computed ONCE; per step only the row shift
     v_t = time_embed[t] @ W1 + b1 changes: h_t = gelu(u + v_t).
  2. The sequential scan is linear in (pred_t, noise_t), so it collapses to
     a weighted sum with host-precomputed scalar weights:
     action = w_init*init + sum_t wp[t]*(h_t @ W2 + b2) + sum_t wn[t]*noise_t
  3. Since W2 is shared across steps, sum_t wp[t]*h_t @ W2 = G @ W2 with
     G = sum_t wp[t]*gelu(u + v_t).  The shifts v_t are tiny (std ~0.02), so
     the 30-term sum over t is replaced by a 2-node quadrature in the shift:
         G[d,b] ~= sum_j c_j[d] * gelu(u[d,b] + mu[d] + x_j)
     matching the 0th/1st moments of {wp[t], v_t[d]}.  Quadrature error
     ~8e-4 relative.
  4. The big matmul u = z @ W1 runs in fp8 e4m3 with DoubleRow perf mode
     (2 contraction tiles per pass -> 2x PE rate).  Raw e4m3 input rounding
     alone gives ~2.2e-2 end-to-end error (over the 2e-2 budget), so a
     CONTROL VARIATE removes the linear component of the error: the device
     computes Gt = G(u8) - lam*u8 with lam[d] = E[G_d'(u)] (host-analytic
     Gauss-Hermite over u ~ N(0, ||W1[:,d]||^2)), and the host adds back the
     exact linear part z @ (W1 * lam) @ W2 into the precomputed noise sum.
     The residual error scales with std(G') not rms(G'), cutting the fp8
     error ~1.5x: measured 1.47e-2 end-to-end.

Per-core schedule (data-parallel over batch, B=16384 -> BL=2048/core):
  Units are (m, h): 128-row tile m of u^T x 1024-column half h.  z arrives
  half-column-major so unit (0,0) only waits on ~1MB.  W1 loads one 256KB
  fp8 DMA per m on the sync queue and stays SBUF-resident.  Per unit:
    PE    : ps[c] += w1[m][:,kk-pair].T @ z8[kk,h][:,:,c*512...]  (DR fp8)
    PE    : po    += w2s[m'].T @ Gt[m',h']                        (1-unit delay)
    ACT   : y_j = gelu(ps*DS + mu + x_j)   (PSUM fp32 in, fp16 out)
    DVE   : Gt = c_0*y_0 + c_1*y_1 + lamc*ps   (lamc = -lam*DS/S)
  out[h-cols] = po + nzT[h-cols] as soon as the last unit of sweep h ends.

fp8 operands are host-quantized (ml_dtypes e4m3 == TRN FP8_EXP4, clip 240):
z scaled by 16, W1 by 512; PSUM holds 8192*u (max ~45k, safely in fp16
range for the early-unit SBUF drains).  A few dummy matmuls up front keep
the PE p-state warm across the DMA ramp.
"""

import sys

import numpy as np

try:
    import concourse  # noqa: F401
except ImportError:
    sys.path.insert(0, "/opt/trn_rl_repo")

import ml_dtypes

import concourse.bass as bass  # noqa: F401
import concourse.tile as tile
from concourse import bacc, mybir
from concourse import bass_utils

F32 = mybir.dt.float32
F16 = mybir.dt.float16
F8 = mybir.dt.float8e4
DR = mybir.MatmulPerfMode.DoubleRow
E4 = ml_dtypes.float8_e4m3

STEPS = 30
B, D, A = 16384, 2048, 64
NCORES = 8
BL = B // NCORES          # 2048 batch rows per core
KT = D // 128             # 16 contraction tiles
KK = KT // 2              # 8 DoubleRow contraction pairs
MT = D // 128             # 16 output-row tiles of u
NB = 512                  # moving-dim chunk (one PSUM bank of fp32)
HB = 1024                 # half-column unit width
NH = BL // HB             # 2 halves
NC = HB // NB             # 2 chunks per half
NODES = (-0.06, 0.06)
NJ = len(NODES)
SZ = 16.0                 # fp8 scale on z
SW = 512.0                # fp8 scale on W1
DS = 1.0 / (SZ * SW)      # PSUM descale


def _schedule_weights():
    """Host constant-folding of the diffusion schedule + scan collapse."""
    t = np.linspace(0.0, STEPS, STEPS + 1) / STEPS
    ab = np.cos((t + 0.008) / 1.008 * np.pi / 2) ** 2
    ab = ab / ab[0]
    beta = np.clip(1.0 - ab[1:] / ab[:-1], 0.0, 0.999)
    alpha = 1.0 - beta
    alpha_bar = np.cumprod(alpha)
    c1 = (1.0 - alpha) / np.sqrt(1.0 - alpha_bar)
    c2 = 1.0 / np.sqrt(alpha)
    c3 = np.sqrt(beta)
    c3[0] = 0.0
    w_init = 1.0
    wp = np.zeros(STEPS)
    wn = np.zeros(STEPS)
    for tt in range(STEPS - 1, -1, -1):  # scan order
        w_init *= c2[tt]
        wp *= c2[tt]
        wn *= c2[tt]
        wp[tt] = -c1[tt] * c2[tt]
        wn[tt] = c3[tt]
    return float(w_init), wp, wn


_W_INIT, _WP, _WN = _schedule_weights()

_PROGRAM = None  # cached compiled Bass program


def _build_program():
    nc = bacc.Bacc("TRN2", target_bir_lowering=False, debug=False,
                   num_devices=NCORES)

    zT_d = nc.dram_tensor("zT", [KK, 128, 2, HB], F8, kind="ExternalInput")
    z1_d = nc.dram_tensor("z1", [KK // 4, 128, 4, 2, HB], F8,
                          kind="ExternalInput")
    w1t_d = nc.dram_tensor("w1t", [MT, 128, KK, 2, 128], F8,
                           kind="ExternalInput")
    w2s_d = nc.dram_tensor("w2s", [128, MT * A], F16, kind="ExternalInput")
    cj_d = nc.dram_tensor("cj", [128, MT * NJ], F32, kind="ExternalInput")
    biasj_d = nc.dram_tensor("biasj", [128, MT * NJ], F32,
                             kind="ExternalInput")
    lam_d = nc.dram_tensor("lam", [128, MT], F32, kind="ExternalInput")
    nzT_d = nc.dram_tensor("nzT", [A, BL], F32, kind="ExternalInput")
    outT_d = nc.dram_tensor("outT", [A, BL], F32, kind="ExternalOutput")

    GELU = mybir.ActivationFunctionType.Gelu
    MUL = mybir.AluOpType.mult
    ADD = mybir.AluOpType.add

    with tile.TileContext(nc) as tc:
        with tc.tile_pool(name="zp", bufs=1) as z_pool, \
             tc.tile_pool(name="w1p", bufs=1) as w1_pool, \
             tc.tile_pool(name="w2p", bufs=1) as w2_pool, \
             tc.tile_pool(name="cjp", bufs=1) as cj_pool, \
             tc.tile_pool(name="yp", bufs=3) as y_pool, \
             tc.tile_pool(name="gp", bufs=4) as g_pool, \
             tc.tile_pool(name="accp", bufs=1) as acc_pool:
            # W1: one 256KB fp8 tile per m, SBUF-resident for both sweeps.
            w1m = [w1_pool.tile([128, KK, 2, 128], F8, tag=f"w1_{m}",
                                name=f"w1_{m}") for m in range(MT)]
            nc.sync.dma_start(w1m[0][:], w1t_d.ap()[0])
            nc.sync.dma_start(w1m[1][:], w1t_d.ap()[1])
            # z h=0 fine-grained on two queues; h=1 as 2 coarse tiles
            zk0 = [z_pool.tile([128, 2, HB], F8, tag=f"z{kk}_0",
                               name=f"zk{kk}_0") for kk in range(KK)]
            zg1 = [z_pool.tile([128, 4, 2, HB], F8, tag=f"z{g}_1",
                               name=f"zg{g}_1") for g in range(KK // 4)]
            for kk in range(KK):
                eng = nc.scalar if kk % 2 == 0 else nc.gpsimd
                eng.dma_start(zk0[kk][:], zT_d.ap()[kk])
            # Bulk prefetch on sync: w1 tiles pace ahead of the ~4.2us unit
            # cadence; z-h1 lands well before the h=1 sweep (~75us in).
            for m in range(2, MT):
                nc.sync.dma_start(w1m[m][:], w1t_d.ap()[m])
            for g in range(KK // 4):
                nc.sync.dma_start(zg1[g][:], z1_d.ap()[g])
            # per-m constants on scalar, needed from the first gelu (~12us)
            bjc = cj_pool.tile([128, MT * NJ], F32, name="bjc")
            cjc = cj_pool.tile([128, MT * NJ], F32, name="cjc")
            lamc = cj_pool.tile([128, MT], F32, name="lamc")
            w2c = w2_pool.tile([128, MT * A], F16, name="w2c")
            nc.scalar.dma_start(bjc[:], biasj_d.ap()[:])
            nc.scalar.dma_start(cjc[:], cj_d.ap()[:])
            nc.scalar.dma_start(lamc[:], lam_d.ap()[:])
            nc.scalar.dma_start(w2c[:], w2s_d.ap()[:])
            nzT = acc_pool.tile([A, BL], F32, name="nzT")
            nc.sync.dma_start(nzT[:], nzT_d.ap()[:])
            acc = acc_pool.tile([A, BL], F32, name="acc")

            def zrhs(kk, h, lo, hi):
                if h == 0:
                    return zk0[kk][:, :, lo:hi]
                return zg1[kk // 4][:, kk % 4, :, lo:hi]

            def w2s_ap(m):
                return w2c[:, m * A:(m + 1) * A]

            with tc.tile_pool(name="pso", bufs=1, space="PSUM") as pso, \
                 tc.tile_pool(name="ps1", bufs=1, space="PSUM") as ps1:
                po = [pso.tile([A, NB], F32, tag=f"po{i}", name=f"po{i}")
                      for i in range(NH * NC)]
                # PE warmup: dependency-free dummy matmuls keep the HAM
                # activity window busy so real matmuls run warm.  The dummy
                # group on po[3] closes with stop=True; the real group
                # re-opens with start=True, which overwrites.
                dum_w = nc.const_aps.tensor(1.0, [128, A],
                                            mybir.dt.bfloat16)
                dum_x = nc.const_aps.tensor(1.0, [128, NB],
                                            mybir.dt.bfloat16)
                NDUM = 8
                for i in range(NDUM):
                    nc.tensor.matmul(po[3][:], dum_w, dum_x,
                                     start=(i == 0), stop=(i == NDUM - 1))

                units = [(m, h) for h in range(NH) for m in range(MT)]
                g_tiles = {}

                def emit_final_mm(m, h):
                    g = g_tiles.pop((m, h))
                    for c in range(NC):
                        nc.tensor.matmul(po[h * NC + c][:], w2s_ap(m),
                                         g[:, c * NB:(c + 1) * NB],
                                         start=(m == 0), stop=(m == MT - 1))

                def emit_out_half(h):
                    csl = slice(h * HB, (h + 1) * HB)
                    for c in range(NC):
                        asl = slice(h * HB + c * NB, h * HB + (c + 1) * NB)
                        nc.vector.tensor_add(acc[:, asl], po[h * NC + c][:],
                                             nzT[:, asl])
                    nc.scalar.dma_start(outT_d.ap()[:, csl], acc[:, csl])

                def emit_sub(c, pc, gl):
                    ml = MT - 1
                    csl = slice(c * NB, (c + 1) * NB)
                    cb = ml * NJ
                    for j in range(NJ):
                        y = y_pool.tile([128, HB], F16, tag="y", name="y")
                        nc.scalar.activation(
                            y[:, csl], pc[:], GELU,
                            bias=bjc[:, cb + j:cb + j + 1], scale=DS)
                        if j == 0:
                            nc.vector.tensor_scalar(
                                gl[:, csl], y[:, csl],
                                cjc[:, cb:cb + 1], None, op0=MUL)
                        else:
                            nc.vector.scalar_tensor_tensor(
                                gl[:, csl], y[:, csl],
                                cjc[:, cb + j:cb + j + 1],
                                gl[:, csl], op0=MUL, op1=ADD)
                    nc.vector.scalar_tensor_tensor(
                        gl[:, csl], pc[:], lamc[:, ml:ml + 1],
                        gl[:, csl], op0=MUL, op1=ADD)
                    nc.tensor.matmul(po[NC + c][:], w2s_ap(ml), gl[:, csl],
                                     start=False, stop=True)
                    asl = slice(HB + c * NB, HB + (c + 1) * NB)
                    nc.vector.tensor_add(acc[:, asl], po[NC + c][:],
                                         nzT[:, asl])
                    nc.scalar.dma_start(outT_d.ap()[:, asl], acc[:, asl])

                unit1_ps = []
                for i, (m, h) in enumerate(units[:-1]):
                    if i == 1:
                        ps = unit1_ps  # computed in the merged loop below
                    else:
                        ps = [ps1.tile([128, NB], F32,
                                       tag=f"pa{(i % 2) * NC + c}",
                                       name=f"ps{c}") for c in range(NC)]
                    if i == 0:
                        # units 0+1 interleaved kk-by-kk: each arriving z
                        # tile feeds 4 matmuls, saturating the PE during
                        # the z-h0 DMA ramp.  Odd kk first: the scalar
                        # queue (odd-kk z) delivers before sync's (which
                        # is behind w1m[0]/w1m[1])
                        unit1_ps = [ps1.tile([128, NB], F32,
                                             tag=f"pa{NC + c}",
                                             name=f"psb{c}")
                                    for c in range(NC)]
                        ks = list(range(KK))
                        for ki, kk in enumerate(ks):
                            for pst, mm in ((ps, 0), (unit1_ps, 1)):
                                for c in range(NC):
                                    nc.tensor.matmul(
                                        pst[c][:],
                                        w1m[mm][:, kk, :, :],
                                        zrhs(kk, 0, c * NB, (c + 1) * NB),
                                        start=(ki == 0),
                                        stop=(ki == KK - 1),
                                        perf_mode=DR)
                        # drain both units' PSUM to SBUF on the idle DVE
                        # so units 2/3 get the banks back without waiting
                        # for the gelu backlog (which reads u16 instead)
                        u16s = [y_pool.tile([128, HB], F16, tag=f"u16{u}",
                                            name=f"u16{u}")
                                for u in range(2)]
                        for pst, u in ((ps, 0), (unit1_ps, 1)):
                            for c in range(NC):
                                nc.vector.tensor_copy(
                                    u16s[u][:, c * NB:(c + 1) * NB],
                                    pst[c][:])
                    elif i >= 2:
                        for kk in range(KK):
                            for c in range(NC):
                                nc.tensor.matmul(
                                    ps[c][:],
                                    w1m[m][:, kk, :, :],
                                    zrhs(kk, h, c * NB, (c + 1) * NB),
                                    start=(kk == 0), stop=(kk == KK - 1),
                                    perf_mode=DR)
                    # finals delayed and BATCHED in pairs: each DR<->f16
                    # mode switch on the PE costs ~250ns of pipeline
                    # bubbles, so emit two units' W2 matmuls per switch
                    if i in (2, 3):
                        fms = {2: [0], 3: [1, 2]}[i]
                    elif i >= 5 and i % 2 == 1:
                        fms = [i - 2, i - 1]
                    else:
                        fms = []
                    for fm in fms:
                        emit_final_mm(*units[fm])
                    if i == MT + 1:  # finals(15,0) done -> h=0 cols complete
                        emit_out_half(0)
                    # gelu reads u straight from PSUM (fp32); units 0/1
                    # read the SBUF drain copy instead (banks recycled).
                    cb = m * NJ
                    g = g_pool.tile([128, HB], F16, tag="g", name="g")
                    for j in range(NJ):
                        y = y_pool.tile([128, HB], F16, tag="y", name="y")
                        for c in range(NC):
                            csl = slice(c * NB, (c + 1) * NB)
                            src = (ps[c][:] if i >= 2
                                   else u16s[i][:, csl])
                            nc.scalar.activation(
                                y[:, csl], src,
                                GELU, bias=bjc[:, cb + j:cb + j + 1],
                                scale=DS)
                        if j == 0:
                            nc.vector.tensor_scalar(g[:], y[:],
                                                    cjc[:, cb:cb + 1],
                                                    None, op0=MUL)
                        else:
                            nc.vector.scalar_tensor_tensor(
                                g[:], y[:], cjc[:, cb + j:cb + j + 1],
                                g[:], op0=MUL, op1=ADD)
                    # control variate: Gt -= lam*u (lamc = -lam*DS/S),
                    # reading the same PSUM banks (or the u16 drain)
                    for c in range(NC):
                        csl = slice(c * NB, (c + 1) * NB)
                        src = (ps[c][:] if i >= 2 else u16s[i][:, csl])
                        nc.vector.scalar_tensor_tensor(
                            g[:, csl], src, lamc[:, m:m + 1],
                            g[:, csl], op0=MUL, op1=ADD)
                    g_tiles[(m, h)] = g

                # last m-tile of the h=1 sweep as two sequential 512-col
                # sub-units: sub-unit c's gelu/combine chain overlaps the
                # other's k-loop, shortening the serial tail to one chunk
                mlast = MT - 1
                gl = g_pool.tile([128, HB], F16, tag="g", name="g")
                pl = []
                for c in range(NC):
                    pc = ps1.tile([128, NB], F32, tag=f"pa{NC + c}",
                                  name=f"psl{c}")
                    pl.append(pc)
                    for kk in range(KK):
                        nc.tensor.matmul(
                            pc[:], w1m[mlast][:, kk, :, :],
                            zrhs(kk, 1, c * NB, (c + 1) * NB),
                            start=(kk == 0), stop=(kk == KK - 1),
                            perf_mode=DR)
                    if c == 0:  # m13/m14's Gt are ready now
                        emit_final_mm(*units[-3])
                        emit_final_mm(*units[-2])
                    else:
                        emit_sub(0, pl[0], gl)
                emit_sub(1, pl[1], gl)

    nc.compile()
    return nc


def _get_program():
    global _PROGRAM
    if _PROGRAM is None:
        _PROGRAM = _build_program()
    return _PROGRAM


def _gelu_prime_mean(sig, b):
    """E[gelu'(x + b)] for x ~ N(0, sig^2), vectorized over rows.

    gelu'(t) = Phi(t) + t*phi(t); 64-pt Gauss-Hermite quadrature."""
    h, wq = np.polynomial.hermite_e.hermegauss(64)
    wq = wq / np.sqrt(2 * np.pi)
    t = sig[:, None] * h[None, :] + b[:, None]
    from scipy.special import erf as _erf
    gp = 0.5 * (1.0 + _erf(t / np.sqrt(2.0))) \
        + t * np.exp(-t * t / 2.0) / np.sqrt(2.0 * np.pi)
    return gp @ wq


def kernel(z, time_embed, W1, b1, W2, b2, init_noise, step_noise,
           _bass_results=None):
    z = np.asarray(z, dtype=np.float32)
    W1 = np.asarray(W1, dtype=np.float32)
    W2 = np.asarray(W2, dtype=np.float32)

    # host precompute: v_t = time_embed @ W1 + b1 (0.1% of total FLOPs)
    V = (np.asarray(time_embed).astype(np.float64) @ W1.astype(np.float64)
         + np.asarray(b1).astype(np.float64))                # [STEPS, D]
    mu = V.mean(axis=0)                                      # [D]
    w = V - mu                                               # centered shifts
    nodes = np.array(NODES, dtype=np.float64)
    vand = np.stack([nodes ** p for p in range(NJ)])         # [NJ, NJ]
    mom = np.stack([np.einsum("t,td->d", _WP, w ** p) for p in range(NJ)])
    c = np.linalg.solve(vand, mom)                           # [NJ, D]
    # normalize Gt's dynamic range into W2 so fp16 Gt stays small
    S = max(1.0, float(np.abs(c).max()) / 8.0)

    # control variate: lam[d] = E[G_d'(u)], u ~ N(0, ||W1[:,d]||^2)
    sig = np.linalg.norm(W1.astype(np.float64), axis=0)      # [D]
    lam = np.zeros(D)
    for j in range(NJ):
        lam += c[j] * _gelu_prime_mean(sig, mu + nodes[j])

    # packed per-m layouts [128, MT*width]: column block m holds rows
    # m*128..(m+1)*128 of the logical [D, width] tensor (1 DMA each)
    def pack(x):  # [D, w] -> [128, MT*w]
        wd = x.shape[1]
        return np.ascontiguousarray(
            x.reshape(MT, 128, wd).transpose(1, 0, 2).reshape(128, MT * wd))

    cj = pack((c / S).T.astype(np.float32).reshape(D, NJ)).astype(np.float32)
    biasj = pack((mu[:, None] + nodes[None, :]).astype(
        np.float32)).astype(np.float32)
    lamc = pack((-lam * DS / S)[:, None].astype(np.float32)).astype(
        np.float32)
    w2s = pack((W2.astype(np.float64) * S)).astype(np.float16)

    # w1t[m, p, kk, i, j] = W1[(2kk+i)*128+p, m*128+j], fp8 e4m3 scaled
    w18 = np.clip(W1.astype(np.float64) * SW, -240.0, 240.0)
    w1t = np.ascontiguousarray(
        w18.reshape(KK, 2, 128, MT, 128).transpose(3, 2, 0, 1, 4).reshape(
            MT, 128, KK, 2, 128)).astype(E4)

    # noise/init/bias weighted sum + the control-variate linear part,
    # all host-side (linear in the inputs)
    nz = _W_INIT * np.asarray(init_noise).astype(np.float64)
    for t in range(STEPS):
        if _WN[t] != 0.0:
            nz += _WN[t] * np.asarray(step_noise[t]).astype(np.float64)
    nz += _WP.sum() * np.asarray(b2).astype(np.float64)      # [B, A]
    # exact linear part that the device's Gt = G - lam*u leaves out:
    # z @ (W1 * lam) @ W2
    M = ((W1.astype(np.float64) * lam[None, :]) @ W2.astype(np.float64))
    nz += z.astype(np.float64) @ M

    z8 = np.clip(z.T.astype(np.float64) * SZ, -240.0, 240.0)  # [D, B]
    z8 = z8.astype(E4)
    nzT = np.ascontiguousarray(nz.T, dtype=np.float32)       # [A, B]
    nc = _get_program()

    in_maps = []
    for cid in range(NCORES):
        bsl = slice(cid * BL, (cid + 1) * BL)
        zc = z8[:, bsl]                                      # [D, BL]
        # h=0: [KK, 128, 2, HB]; z[(2kk+i)*128+p, col]
        zh0 = np.ascontiguousarray(
            zc[:, 0:HB].reshape(KK, 2, 128, HB).transpose(0, 2, 1, 3))
        # h=1 packed as KK/4 groups of 4 kk side by side
        z1 = np.ascontiguousarray(
            zc[:, HB:].reshape(KK // 4, 4, 2, 128, HB).transpose(
                0, 3, 1, 2, 4))
        in_maps.append({
            "zT": zh0,
            "z1": z1,
            "w1t": w1t,
            "w2s": w2s,
            "cj": cj,
            "biasj": biasj,
            "lam": lamc,
            "nzT": np.ascontiguousarray(nzT[:, bsl]),
        })

    res = bass_utils.run_bass_kernel_spmd(
        nc, in_maps, core_ids=list(range(NCORES)))
    if _bass_results is not None:
        _bass_results.append(res)

    out = np.empty((B, A), dtype=np.float32)
    for cid in range(NCORES):
        out[cid * BL:(cid + 1) * BL] = res.results[cid]["outT"].T
    return out


# revision 13
# speedup vs baseline: 1.1110x; 1.1110x over previous
"""Trainium2 Bass kernel for nn_DiffusionNetwork (30-step diffusion sampling).

Algorithm (algebraic restructuring + quadrature collapse + fp8 control
variate):
  1. The MLP input ``cond = z + time_embed[t]`` is independent of the scanned
     ``action``, so u = z @ W1 is computed ONCE; per step only the row shift
     v_t = time_embed[t] @ W1 + b1 changes: h_t = gelu(u + v_t).
  2. The sequential scan is linear in (pred_t, noise_t), so it collapses to
     a weighted sum with host-precomputed scalar weights:
     action = w_init*init + sum_t wp[t]*(h_t @ W2 + b2) + sum_t wn[t]*noise_t
  3. Since W2 is shared across steps, sum_t wp[t]*h_t @ W2 = G @ W2 with
     G = sum_t wp[t]*gelu(u + v_t).  The shifts v_t are tiny (std ~0.02), so
     the 30-term sum over t is replaced by a 2-node quadrature in the shift:
         G[d,b] ~= sum_j c_j[d] * gelu(u[d,b] + mu[d] + x_j)
     matching the 0th/1st moments of {wp[t], v_t[d]}.  Quadrature error
     ~8e-4 relative.
  4. The big matmul u = z @ W1 runs in fp8 e4m3 with DoubleRow perf mode
     (2 contraction tiles per pass -> 2x PE rate).  Raw e4m3 input rounding
     alone gives ~2.2e-2 end-to-end error (over the 2e-2 budget), so a
     CONTROL VARIATE removes the linear component of the error: the device
     computes Gt = G(u8) - lam*u8 with lam[d] = E[G_d'(u)] (host-analytic
     Gauss-Hermite over u ~ N(0, ||W1[:,d]||^2)), and the host adds back the
     exact linear part z @ (W1 * lam) @ W2 into the precomputed noise sum.
     The residual error scales with std(G') not rms(G'), cutting the fp8
     error ~1.5x: measured 1.47e-2 end-to-end.

Per-core schedule (data-parallel over batch, B=16384 -> BL=2048/core):
  Units are (m, h): 128-row tile m of u^T x 1024-column half h.  z arrives
  half-column-major so unit (0,0) only waits on ~1MB.  W1 loads one 256KB
  fp8 DMA per m on the sync queue and stays SBUF-resident.  Per unit:
    PE    : ps[c] += w1[m][:,kk-pair].T @ z8[kk,h][:,:,c*512...]  (DR fp8)
    PE    : po    += w2s[m'].T @ Gt[m',h']                        (1-unit delay)
    ACT   : y_j = gelu(ps*DS + mu + x_j)   (PSUM fp32 in, fp16 out)
    DVE   : Gt = c_0*y_0 + c_1*y_1 + lamc*ps   (lamc = -lam*DS/S)
  out[h-cols] = po + nzT[h-cols] as soon as the last unit of sweep h ends.

fp8 operands are host-quantized (ml_dtypes e4m3 == TRN FP8_EXP4, clip 240):
z scaled by 16, W1 by 512; PSUM holds 8192*u (max ~45k, safely in fp16
range for the early-unit SBUF drains).  A few dummy matmuls up front keep
the PE p-state warm across the DMA ramp.
"""

import sys

import numpy as np

try:
    import concourse  # noqa: F401
except ImportError:
    sys.path.insert(0, "/opt/trn_rl_repo")

import ml_dtypes

import concourse.bass as bass  # noqa: F401
import concourse.tile as tile
from concourse import bacc, mybir
from concourse import bass_utils

F32 = mybir.dt.float32
F16 = mybir.dt.float16
F8 = mybir.dt.float8e4
DR = mybir.MatmulPerfMode.DoubleRow
E4 = ml_dtypes.float8_e4m3

STEPS = 30
B, D, A = 16384, 2048, 64
NCORES = 8
BL = B // NCORES          # 2048 batch rows per core
KT = D // 128             # 16 contraction tiles
KK = KT // 2              # 8 DoubleRow contraction pairs
MT = D // 128             # 16 output-row tiles of u
NB = 512                  # moving-dim chunk (one PSUM bank of fp32)
HB = 1024                 # half-column unit width
NH = BL // HB             # 2 halves
NC = HB // NB             # 2 chunks per half
NODES = (-0.06, 0.06)
NJ = len(NODES)
SZ = 16.0                 # fp8 scale on z
SW = 512.0                # fp8 scale on W1
DS = 1.0 / (SZ * SW)      # PSUM descale


def _schedule_weights():
    """Host constant-folding of the diffusion schedule + scan collapse."""
    t = np.linspace(0.0, STEPS, STEPS + 1) / STEPS
    ab = np.cos((t + 0.008) / 1.008 * np.pi / 2) ** 2
    ab = ab / ab[0]
    beta = np.clip(1.0 - ab[1:] / ab[:-1], 0.0, 0.999)
    alpha = 1.0 - beta
    alpha_bar = np.cumprod(alpha)
    c1 = (1.0 - alpha) / np.sqrt(1.0 - alpha_bar)
    c2 = 1.0 / np.sqrt(alpha)
    c3 = np.sqrt(beta)
    c3[0] = 0.0
    w_init = 1.0
    wp = np.zeros(STEPS)
    wn = np.zeros(STEPS)
    for tt in range(STEPS - 1, -1, -1):  # scan order
        w_init *= c2[tt]
        wp *= c2[tt]
        wn *= c2[tt]
        wp[tt] = -c1[tt] * c2[tt]
        wn[tt] = c3[tt]
    return float(w_init), wp, wn


_W_INIT, _WP, _WN = _schedule_weights()

_PROGRAM = None  # cached compiled Bass program


def _build_program():
    nc = bacc.Bacc("TRN2", target_bir_lowering=False, debug=False,
                   num_devices=NCORES)

    zT_d = nc.dram_tensor("zT", [KK, 128, 2, HB], F8, kind="ExternalInput")
    z1_d = nc.dram_tensor("z1", [KK // 4, 128, 4, 2, HB], F8,
                          kind="ExternalInput")
    w1t_d = nc.dram_tensor("w1t", [MT, 128, KK, 2, 128], F8,
                           kind="ExternalInput")
    w2s_d = nc.dram_tensor("w2s", [128, MT * A], F16, kind="ExternalInput")
    cj_d = nc.dram_tensor("cj", [128, MT * NJ], F32, kind="ExternalInput")
    biasj_d = nc.dram_tensor("biasj", [128, MT * NJ], F32,
                             kind="ExternalInput")
    lam_d = nc.dram_tensor("lam", [128, MT], F32, kind="ExternalInput")
    nzT_d = nc.dram_tensor("nzT", [A, BL], F32, kind="ExternalInput")
    outT_d = nc.dram_tensor("outT", [A, BL], F32, kind="ExternalOutput")

    GELU = mybir.ActivationFunctionType.Gelu
    MUL = mybir.AluOpType.mult
    ADD = mybir.AluOpType.add

    with tile.TileContext(nc) as tc:
        with tc.tile_pool(name="zp", bufs=1) as z_pool, \
             tc.tile_pool(name="w1p", bufs=1) as w1_pool, \
             tc.tile_pool(name="w2p", bufs=1) as w2_pool, \
             tc.tile_pool(name="cjp", bufs=1) as cj_pool, \
             tc.tile_pool(name="yp", bufs=3) as y_pool, \
             tc.tile_pool(name="gp", bufs=4) as g_pool, \
             tc.tile_pool(name="accp", bufs=1) as acc_pool:
            # W1: one 256KB fp8 tile per m, SBUF-resident for both sweeps.
            w1m = [w1_pool.tile([128, KK, 2, 128], F8, tag=f"w1_{m}",
                                name=f"w1_{m}") for m in range(MT)]
            nc.sync.dma_start(w1m[0][:], w1t_d.ap()[0])
            nc.sync.dma_start(w1m[1][:], w1t_d.ap()[1])
            # z h=0 fine-grained on two queues; h=1 as 2 coarse tiles
            zk0 = [z_pool.tile([128, 2, HB], F8, tag=f"z{kk}_0",
                               name=f"zk{kk}_0") for kk in range(KK)]
            zg1 = [z_pool.tile([128, 4, 2, HB], F8, tag=f"z{g}_1",
                               name=f"zg{g}_1") for g in range(KK // 4)]
            for kk in range(KK):
                eng = nc.scalar if kk % 2 == 0 else nc.gpsimd
                eng.dma_start(zk0[kk][:], zT_d.ap()[kk])
            # Bulk prefetch on sync: w1 tiles pace ahead of the ~4.2us unit
            # cadence; z-h1 lands well before the h=1 sweep (~75us in).
            for m in range(2, MT):
                nc.sync.dma_start(w1m[m][:], w1t_d.ap()[m])
            for g in range(KK // 4):
                nc.sync.dma_start(zg1[g][:], z1_d.ap()[g])
            # per-m constants on scalar, needed from the first gelu (~12us)
            bjc = cj_pool.tile([128, MT * NJ], F32, name="bjc")
            cjc = cj_pool.tile([128, MT * NJ], F32, name="cjc")
            lamc = cj_pool.tile([128, MT], F32, name="lamc")
            w2c = w2_pool.tile([128, MT * A], F16, name="w2c")
            nc.scalar.dma_start(bjc[:], biasj_d.ap()[:])
            nc.scalar.dma_start(cjc[:], cj_d.ap()[:])
            nc.scalar.dma_start(lamc[:], lam_d.ap()[:])
            nc.scalar.dma_start(w2c[:], w2s_d.ap()[:])
            nzT = acc_pool.tile([A, BL], F32, name="nzT")
            nc.sync.dma_start(nzT[:], nzT_d.ap()[:])
            acc = acc_pool.tile([A, BL], F32, name="acc")

            def zrhs(kk, h, lo, hi):
                if h == 0:
                    return zk0[kk][:, :, lo:hi]
                return zg1[kk // 4][:, kk % 4, :, lo:hi]

            def w2s_ap(m):
                return w2c[:, m * A:(m + 1) * A]

            with tc.tile_pool(name="pso", bufs=1, space="PSUM") as pso, \
                 tc.tile_pool(name="ps1", bufs=1, space="PSUM") as ps1:
                po = [pso.tile([A, NB], F32, tag=f"po{i}", name=f"po{i}")
                      for i in range(NH * NC)]
                # PE warmup: dependency-free dummy matmuls keep the HAM
                # activity window busy so real matmuls run warm.  The dummy
                # group on po[3] closes with stop=True; the real group
                # re-opens with start=True, which overwrites.
                dum_w = nc.const_aps.tensor(1.0, [128, A],
                                            mybir.dt.bfloat16)
                dum_x = nc.const_aps.tensor(1.0, [128, NB],
                                            mybir.dt.bfloat16)
                NDUM = 8
                for i in range(NDUM):
                    nc.tensor.matmul(po[3][:], dum_w, dum_x,
                                     start=(i == 0), stop=(i == NDUM - 1))

                units = [(m, h) for h in range(NH) for m in range(MT)]
                g_tiles = {}

                def emit_final_mm(m, h):
                    g = g_tiles.pop((m, h))
                    for c in range(NC):
                        nc.tensor.matmul(po[h * NC + c][:], w2s_ap(m),
                                         g[:, c * NB:(c + 1) * NB],
                                         start=(m == 0), stop=(m == MT - 1))

                def emit_out_half(h):
                    csl = slice(h * HB, (h + 1) * HB)
                    for c in range(NC):
                        asl = slice(h * HB + c * NB, h * HB + (c + 1) * NB)
                        nc.vector.tensor_add(acc[:, asl], po[h * NC + c][:],
                                             nzT[:, asl])
                    nc.scalar.dma_start(outT_d.ap()[:, csl], acc[:, csl])

                def emit_sub(c, pc, gl):
                    ml = MT - 1
                    csl = slice(c * NB, (c + 1) * NB)
                    cb = ml * NJ
                    nc.vector.tensor_scalar(
                        gl[:, csl], pc[:], lamc[:, ml:ml + 1], None,
                        op0=MUL)
                    for j in range(NJ):
                        y = y_pool.tile([128, HB], F16, tag="y", name="y")
                        nc.scalar.activation(
                            y[:, csl], pc[:], GELU,
                            bias=bjc[:, cb + j:cb + j + 1], scale=DS)
                        nc.vector.scalar_tensor_tensor(
                            gl[:, csl], y[:, csl],
                            cjc[:, cb + j:cb + j + 1],
                            gl[:, csl], op0=MUL, op1=ADD)
                    nc.tensor.matmul(po[NC + c][:], w2s_ap(ml), gl[:, csl],
                                     start=False, stop=True)
                    asl = slice(HB + c * NB, HB + (c + 1) * NB)
                    nc.vector.tensor_add(acc[:, asl], po[NC + c][:],
                                         nzT[:, asl])
                    nc.scalar.dma_start(outT_d.ap()[:, asl], acc[:, asl])

                unit1_ps = []
                for i, (m, h) in enumerate(units[:-1]):
                    if i == 1:
                        ps = unit1_ps  # computed in the merged loop below
                    else:
                        ps = [ps1.tile([128, NB], F32,
                                       tag=f"pa{(i % 2) * NC + c}",
                                       name=f"ps{c}") for c in range(NC)]
                    if i == 0:
                        # units 0+1 interleaved kk-by-kk: each arriving z
                        # tile feeds 4 matmuls, saturating the PE during
                        # the z-h0 DMA ramp.  Odd kk first: the scalar
                        # queue (odd-kk z) delivers before sync's (which
                        # is behind w1m[0]/w1m[1])
                        unit1_ps = [ps1.tile([128, NB], F32,
                                             tag=f"pa{NC + c}",
                                             name=f"psb{c}")
                                    for c in range(NC)]
                        ks = list(range(KK))
                        for ki, kk in enumerate(ks):
                            for pst, mm in ((ps, 0), (unit1_ps, 1)):
                                for c in range(NC):
                                    nc.tensor.matmul(
                                        pst[c][:],
                                        w1m[mm][:, kk, :, :],
                                        zrhs(kk, 0, c * NB, (c + 1) * NB),
                                        start=(ki == 0),
                                        stop=(ki == KK - 1),
                                        perf_mode=DR)
                        # drain both units' PSUM to SBUF on the idle DVE
                        # so units 2/3 get the banks back without waiting
                        # for the gelu backlog (which reads u16 instead)
                        u16s = [y_pool.tile([128, HB], F16, tag=f"u16{u}",
                                            name=f"u16{u}")
                                for u in range(2)]
                        for pst, u in ((ps, 0), (unit1_ps, 1)):
                            for c in range(NC):
                                nc.vector.tensor_copy(
                                    u16s[u][:, c * NB:(c + 1) * NB],
                                    pst[c][:])
                    elif i >= 2:
                        for kk in range(KK):
                            for c in range(NC):
                                nc.tensor.matmul(
                                    ps[c][:],
                                    w1m[m][:, kk, :, :],
                                    zrhs(kk, h, c * NB, (c + 1) * NB),
                                    start=(kk == 0), stop=(kk == KK - 1),
                                    perf_mode=DR)
                    # finals BATCHED in pairs, two units late: each
                    # DR<->f16 mode switch on the PE costs ~250ns of
                    # pipeline bubbles, so emit two units' W2 matmuls per
                    # switch, and only once their Gt (whose DVE chain ends
                    # ~4us after the unit's k-loop) is certainly done
                    if i == 3:
                        fms = [0, 1]
                    elif i >= 5 and i % 2 == 1:
                        fms = [i - 3, i - 2]
                    else:
                        fms = []
                    for fm in fms:
                        emit_final_mm(*units[fm])
                    if i == MT + 1:  # finals(15,0) done -> h=0 cols complete
                        emit_out_half(0)
                    # gelu reads u straight from PSUM (fp32); units 0/1
                    # read the SBUF drain copy instead (banks recycled).
                    # The control variate Gt = -lam*u runs FIRST on the
                    # DVE (it only needs PSUM, ready at k-loop end) so Gt
                    # completes right after the last gelu combine instead
                    # of ~1.5us later.
                    cb = m * NJ
                    g = g_pool.tile([128, HB], F16, tag="g", name="g")
                    for c in range(NC):
                        csl = slice(c * NB, (c + 1) * NB)
                        src = (ps[c][:] if i >= 2 else u16s[i][:, csl])
                        nc.vector.tensor_scalar(
                            g[:, csl], src, lamc[:, m:m + 1], None,
                            op0=MUL)
                    for j in range(NJ):
                        y = y_pool.tile([128, HB], F16, tag="y", name="y")
                        for c in range(NC):
                            csl = slice(c * NB, (c + 1) * NB)
                            src = (ps[c][:] if i >= 2
                                   else u16s[i][:, csl])
                            nc.scalar.activation(
                                y[:, csl], src,
                                GELU, bias=bjc[:, cb + j:cb + j + 1],
                                scale=DS)
                        nc.vector.scalar_tensor_tensor(
                            g[:], y[:], cjc[:, cb + j:cb + j + 1],
                            g[:], op0=MUL, op1=ADD)
                    g_tiles[(m, h)] = g

                # last m-tile of the h=1 sweep as two sequential 512-col
                # sub-units: sub-unit c's gelu/combine chain overlaps the
                # other's k-loop, shortening the serial tail to one chunk
                mlast = MT - 1
                gl = g_pool.tile([128, HB], F16, tag="g", name="g")
                pl = []
                for c in range(NC):
                    pc = ps1.tile([128, NB], F32, tag=f"pa{NC + c}",
                                  name=f"psl{c}")
                    pl.append(pc)
                    for kk in range(KK):
                        nc.tensor.matmul(
                            pc[:], w1m[mlast][:, kk, :, :],
                            zrhs(kk, 1, c * NB, (c + 1) * NB),
                            start=(kk == 0), stop=(kk == KK - 1),
                            perf_mode=DR)
                    if c == 0:  # m12..m14's Gt are ready now
                        emit_final_mm(*units[-4])
                        emit_final_mm(*units[-3])
                        emit_final_mm(*units[-2])
                    else:
                        emit_sub(0, pl[0], gl)
                emit_sub(1, pl[1], gl)

    nc.compile()
    return nc


def _get_program():
    global _PROGRAM
    if _PROGRAM is None:
        _PROGRAM = _build_program()
    return _PROGRAM


def _gelu_prime_mean(sig, b):
    """E[gelu'(x + b)] for x ~ N(0, sig^2), vectorized over rows.

    gelu'(t) = Phi(t) + t*phi(t); 64-pt Gauss-Hermite quadrature."""
    h, wq = np.polynomial.hermite_e.hermegauss(64)
    wq = wq / np.sqrt(2 * np.pi)
    t = sig[:, None] * h[None, :] + b[:, None]
    from scipy.special import erf as _erf
    gp = 0.5 * (1.0 + _erf(t / np.sqrt(2.0))) \
        + t * np.exp(-t * t / 2.0) / np.sqrt(2.0 * np.pi)
    return gp @ wq


def kernel(z, time_embed, W1, b1, W2, b2, init_noise, step_noise,
           _bass_results=None):
    z = np.asarray(z, dtype=np.float32)
    W1 = np.asarray(W1, dtype=np.float32)
    W2 = np.asarray(W2, dtype=np.float32)

    # host precompute: v_t = time_embed @ W1 + b1 (0.1% of total FLOPs)
    V = (np.asarray(time_embed).astype(np.float64) @ W1.astype(np.float64)
         + np.asarray(b1).astype(np.float64))                # [STEPS, D]
    mu = V.mean(axis=0)                                      # [D]
    w = V - mu                                               # centered shifts
    nodes = np.array(NODES, dtype=np.float64)
    vand = np.stack([nodes ** p for p in range(NJ)])         # [NJ, NJ]
    mom = np.stack([np.einsum("t,td->d", _WP, w ** p) for p in range(NJ)])
    c = np.linalg.solve(vand, mom)                           # [NJ, D]
    # normalize Gt's dynamic range into W2 so fp16 Gt stays small
    S = max(1.0, float(np.abs(c).max()) / 8.0)

    # control variate: lam[d] = E[G_d'(u)], u ~ N(0, ||W1[:,d]||^2)
    sig = np.linalg.norm(W1.astype(np.float64), axis=0)      # [D]
    lam = np.zeros(D)
    for j in range(NJ):
        lam += c[j] * _gelu_prime_mean(sig, mu + nodes[j])

    # packed per-m layouts [128, MT*width]: column block m holds rows
    # m*128..(m+1)*128 of the logical [D, width] tensor (1 DMA each)
    def pack(x):  # [D, w] -> [128, MT*w]
        wd = x.shape[1]
        return np.ascontiguousarray(
            x.reshape(MT, 128, wd).transpose(1, 0, 2).reshape(128, MT * wd))

    cj = pack((c / S).T.astype(np.float32).reshape(D, NJ)).astype(np.float32)
    biasj = pack((mu[:, None] + nodes[None, :]).astype(
        np.float32)).astype(np.float32)
    lamc = pack((-lam * DS / S)[:, None].astype(np.float32)).astype(
        np.float32)
    w2s = pack((W2.astype(np.float64) * S)).astype(np.float16)

    # w1t[m, p, kk, i, j] = W1[(2kk+i)*128+p, m*128+j], fp8 e4m3 scaled
    w18 = np.clip(W1.astype(np.float64) * SW, -240.0, 240.0)
    w1t = np.ascontiguousarray(
        w18.reshape(KK, 2, 128, MT, 128).transpose(3, 2, 0, 1, 4).reshape(
            MT, 128, KK, 2, 128)).astype(E4)

    # noise/init/bias weighted sum + the control-variate linear part,
    # all host-side (linear in the inputs)
    nz = _W_INIT * np.asarray(init_noise).astype(np.float64)
    for t in range(STEPS):
        if _WN[t] != 0.0:
            nz += _WN[t] * np.asarray(step_noise[t]).astype(np.float64)
    nz += _WP.sum() * np.asarray(b2).astype(np.float64)      # [B, A]
    # exact linear part that the device's Gt = G - lam*u leaves out:
    # z @ (W1 * lam) @ W2
    M = ((W1.astype(np.float64) * lam[None, :]) @ W2.astype(np.float64))
    nz += z.astype(np.float64) @ M

    z8 = np.clip(z.T.astype(np.float64) * SZ, -240.0, 240.0)  # [D, B]
    z8 = z8.astype(E4)
    nzT = np.ascontiguousarray(nz.T, dtype=np.float32)       # [A, B]
    nc = _get_program()

    in_maps = []
    for cid in range(NCORES):
        bsl = slice(cid * BL, (cid + 1) * BL)
        zc = z8[:, bsl]                                      # [D, BL]
        # h=0: [KK, 128, 2, HB]; z[(2kk+i)*128+p, col]
        zh0 = np.ascontiguousarray(
            zc[:, 0:HB].reshape(KK, 2, 128, HB).transpose(0, 2, 1, 3))
        # h=1 packed as KK/4 groups of 4 kk side by side
        z1 = np.ascontiguousarray(
            zc[:, HB:].reshape(KK // 4, 4, 2, 128, HB).transpose(
                0, 3, 1, 2, 4))
        in_maps.append({
            "zT": zh0,
            "z1": z1,
            "w1t": w1t,
            "w2s": w2s,
            "cj": cj,
            "biasj": biasj,
            "lam": lamc,
            "nzT": np.ascontiguousarray(nzT[:, bsl]),
        })

    res = bass_utils.run_bass_kernel_spmd(
        nc, in_maps, core_ids=list(range(NCORES)))
    if _bass_results is not None:
        _bass_results.append(res)

    out = np.empty((B, A), dtype=np.float32)
    for cid in range(NCORES):
        out[cid * BL:(cid + 1) * BL] = res.results[cid]["outT"].T
    return out
